# revision 24
# baseline (speedup 1.0000x reference)
"""Block-quantized FP8 linear (KLinearFP8) on 8 trn2 NeuronCores.

y[m, n] = sum_k x_dq[m, k] * w_dq[n, k]
  x_dq: per-(row, 128-block) fp8e4m3fn-simulated quantization of x
  w_dq: weight (fp8 values held in fp32) * per-128x128-block scale

Sharding: column-parallel. weight/weight_scale_inv split along N across 8
cores, x replicated; each core computes y[:, c*2048:(c+1)*2048].

Per-core kernel: dequantize both operands to bf16 on-chip (TRN e4m3 max is
240 vs OCP's 448, so x is quantized with scale amax/224 — a power-of-two
rescale of the reference's amax/448 grid, giving identical rounding), then
a k-on-partitions bf16 GEMM with fp32 PSUM accumulation.

The PE runs only GEMM matmuls (~884us roofline for this shape): weight
tiles are transposed by XBAR dma_start_transpose instead of the PE, all
on the sync ring (transposes from two HWDGE rings concurrently corrupt
data on HW).  Weight dequant-scale mults alternate between gpsimd and
vector (gpsimd alone is ~2.5x slower than the HBM delivers pieces).  The
first two m-tiles run chunk-major at half-K granularity so the matmul
stream starts early and stays dense (keeps HAM warm) while the 32MB
weight read streams in; all 8 PSUM banks rotate through 512-wide
accumulation groups.
"""

import numpy as np

M, K, N = 4096, 4096, 16384
NCORES = 8
NSH = N // NCORES          # 2048 columns of y per core
P = 128
KB = K // P                # 32 k-blocks
NB = NSH // P              # 16 n-blocks per core
FP8_SAFE = 224.0           # 448/2: fits TRN e4m3 (max 240), same rounding grid

_NC_CACHE = {}


def _build(M=M, K=K, NSH=NSH, debug=False):
    import concourse.bass as bass  # noqa: F401
    import concourse.mybir as mybir
    import concourse.tile as tile
    from concourse import bacc

    KB = K // P                # k-blocks
    KH = KB // 2               # k-blocks per half
    MT = M // P                # m-tiles
    NB = NSH // P              # n-blocks
    CHW = min(512, NSH)        # psum chunk width
    NCH = NSH // CHW           # chunks per core
    NPC = CHW // P             # n-blocks per chunk

    f32, bf16, f8 = mybir.dt.float32, mybir.dt.bfloat16, mybir.dt.float8e4

    nc = bacc.Bacc(None, target_bir_lowering=False, debug=debug)
    x_d = nc.declare_dram_parameter("x", [M, K], f32, isOutput=False)
    w_d = nc.declare_dram_parameter("w", [NSH, K], f32, isOutput=False)
    ws_d = nc.declare_dram_parameter("ws", [NB, KB], f32, isOutput=False)
    y_d = nc.declare_dram_parameter("y", [M, NSH], f32, isOutput=True)

    with tile.TileContext(nc) as tc:
        with (
            tc.tile_pool(name="const", bufs=1) as const,
            tc.tile_pool(name="wt", bufs=1) as wtp,
            tc.tile_pool(name="wdq", bufs=2) as wpool,
            tc.tile_pool(name="xrow", bufs=2) as xpool,
            tc.tile_pool(name="xq", bufs=2) as xqp,
            tc.tile_pool(name="xdq", bufs=2) as xdp,
            tc.tile_pool(name="xt", bufs=4) as xtp,
            tc.tile_pool(name="scales", bufs=3) as spool,
            tc.tile_pool(name="ypool", bufs=3) as ypool,
            tc.tile_pool(name="psum", bufs=8, space="PSUM") as psum,
        ):
            # ---- weight-block scales, broadcast to all partitions ----
            ws_row = const.tile([1, NB * KB], f32)
            nc.sync.dma_start(ws_row[:], ws_d[:].rearrange("a b -> (a b)")[None, :])
            ws_b = const.tile([P, NB, KB], f32)
            nc.gpsimd.partition_broadcast(
                ws_b[:].rearrange("p a b -> p (a b)"), ws_row[:]
            )

            # Persistent transposed weights: [k-part, nb, kb, n].  For chunk c
            # the matmul streams wT[:, c*NPC:(c+1)*NPC, kb, :] (3D strided AP);
            # each XBAR transpose destination wT[:, nb, khalf, :] is contiguous.
            wT = wtp.tile([P, NB, KB, P], bf16)

            def w_piece(nb, kh):
                # fp32->bf16 cast in-flight (SWDGE). Exact: weight holds
                # fp8-representable values.  Scale mults alternate between
                # gpsimd and vector; transpose on the sync-ring XBAR.
                ks = slice(kh * KH * P, (kh + 1) * KH * P)
                wdq = wpool.tile([P, KH, P], bf16, tag="wdq")
                nc.gpsimd.dma_start(
                    wdq[:],
                    w_d[nb * P:(nb + 1) * P, ks].rearrange(
                        "n (kb x) -> n kb x", x=P
                    ),
                )
                eng = nc.vector if nb % 2 == 0 else nc.gpsimd
                eng.tensor_tensor(
                    wdq[:], wdq[:],
                    ws_b[:, nb, kh * KH:(kh + 1) * KH, None].to_broadcast(
                        (P, KH, P)
                    ),
                    mybir.AluOpType.mult,
                )
                nc.sync.dma_start_transpose(
                    wT[:, nb, kh * KH:(kh + 1) * KH, :],
                    wdq[:].rearrange("p a b -> p (a b)"),
                )

            xts = {}

            def x_prep(mt):
                # quantize+dequantize one m-tile of x (two k-halves) onto the
                # reference fp8 grid, then XBAR-transpose to k-on-partitions.
                ms = slice(mt * P, (mt + 1) * P)
                sc = spool.tile([P, 3, KB], f32, tag="sc")
                amax, rinv, s2 = sc[:, 0, :], sc[:, 1, :], sc[:, 2, :]
                xrows = []
                for kh in range(2):
                    ks = slice(kh * KH * P, (kh + 1) * KH * P)
                    xrow = xpool.tile([P, KH, P], f32, tag="xrow")
                    nc.sync.dma_start(
                        xrow[:],
                        x_d[ms, ks].rearrange("m (kb x) -> m kb x", x=P),
                    )
                    nc.vector.tensor_reduce(
                        amax[:, kh * KH:(kh + 1) * KH], xrow[:],
                        axis=mybir.AxisListType.X,
                        op=mybir.AluOpType.max, apply_absolute_value=True,
                    )
                    xrows.append(xrow)
                nc.vector.reciprocal(rinv, amax)
                nc.vector.tensor_scalar_mul(rinv, rinv, float(FP8_SAFE))
                nc.vector.tensor_scalar_mul(s2, amax, float(1.0 / FP8_SAFE))
                xT = xtp.tile([P, KB, P], bf16, tag="xT")
                for kh in range(2):
                    khs = slice(kh * KH, (kh + 1) * KH)
                    xq = xqp.tile([P, KH, P], f8, tag="xq")
                    nc.vector.tensor_tensor(
                        xq[:], xrows[kh][:],
                        rinv[:, khs, None].to_broadcast((P, KH, P)),
                        mybir.AluOpType.mult,
                    )
                    xdq = xdp.tile([P, KH, P], bf16, tag="xdq")
                    nc.vector.tensor_tensor(
                        xdq[:], xq[:],
                        s2[:, khs, None].to_broadcast((P, KH, P)),
                        mybir.AluOpType.mult,
                    )
                    nc.sync.dma_start_transpose(
                        xT[:, khs, :], xdq[:].rearrange("p a b -> p (a b)")
                    )
                xts[mt] = xT

            pts = {}

            def half_job(mt, c, kh):
                # 16 matmuls: psum[mt,c] += x[mt, khalf].T @ w[chunk c, khalf]
                if kh == 0:
                    pts[(mt, c)] = psum.tile(
                        [P, CHW], mybir.dt.float32, name=f"pt{mt}_{c}", tag="pt"
                    )
                pt = pts[(mt, c)]
                xT = xts[mt]
                for kb in range(kh * KH, (kh + 1) * KH):
                    nc.tensor.matmul(
                        pt[:],
                        xT[:, kb, :],
                        wT[:, c * NPC:(c + 1) * NPC, kb, :],
                        start=(kb == 0),
                        stop=(kb == KB - 1),
                    )

            def drain(mt, c):
                pt = pts.pop((mt, c))
                yt = ypool.tile([P, CHW], mybir.dt.float32, tag="yt")
                if c % 2 == 0:
                    nc.scalar.activation(
                        yt[:], pt[:], mybir.ActivationFunctionType.Copy
                    )
                else:
                    nc.vector.tensor_copy(yt[:], pt[:])
                nc.scalar.dma_start(
                    y_d[mt * P:(mt + 1) * P, c * CHW:(c + 1) * CHW], yt[:]
                )

            # ---- emission ----
            # Early weights (chunk 0 first half) so matmuls can start ASAP,
            # interleaved with x-prep for the ramp m-tiles.
            R = min(2, MT)          # ramp m-tiles, processed chunk-major
            NPREP = min(4, MT)      # m-tiles prepped ahead of the job stream
            for nb in range(min(NPC, NB)):
                w_piece(nb, 0)
            for mt in range(min(2, NPREP)):
                x_prep(mt)
            for nb in range(min(NPC, NB)):
                w_piece(nb, 1)
            for mt in range(2, NPREP):
                x_prep(mt)
            for c in range(1, NCH):
                for kh in range(2):
                    for nb in range(c * NPC, (c + 1) * NPC):
                        w_piece(nb, kh)

            # Ramp: chunk-major at half-K granularity over the first R m-tiles.
            for c in range(NCH):
                for kh in range(2):
                    for mt in range(R):
                        half_job(mt, c, kh)
                        if kh == 1:
                            drain(mt, c)

            # Steady state: m-tile-major.
            for mt in range(R, MT):
                if mt + 2 < MT and mt + 2 >= NPREP:
                    x_prep(mt + 2)
                for c in range(NCH):
                    for kh in range(2):
                        half_job(mt, c, kh)
                    drain(mt, c)

    nc.compile()
    return nc


def kernel(x, weight, weight_scale_inv):
    from concourse.bass_utils import run_bass_kernel_spmd

    if "nc" not in _NC_CACHE:
        _NC_CACHE["nc"] = _build()
    nc = _NC_CACHE["nc"]

    x = np.ascontiguousarray(np.asarray(x, dtype=np.float32))
    weight = np.asarray(weight, dtype=np.float32)
    ws = np.asarray(weight_scale_inv, dtype=np.float32)

    in_maps = [
        {
            "x": x,
            "w": np.ascontiguousarray(weight[c * NSH:(c + 1) * NSH]),
            "ws": np.ascontiguousarray(ws[c * NB:(c + 1) * NB]),
        }
        for c in range(NCORES)
    ]
    res = run_bass_kernel_spmd(nc, in_maps, list(range(NCORES)))
    y = np.concatenate([res.results[c]["y"] for c in range(NCORES)], axis=1)
    return y.astype(np.float32, copy=False)


# revision 27
# speedup vs baseline: 1.1554x; 1.1554x over previous
"""Block-quantized FP8 linear (KLinearFP8) on 8 trn2 NeuronCores.

y[m, n] = sum_k x_dq[m, k] * w_dq[n, k]
  x_dq: per-(row, 128-block) fp8e4m3fn-simulated quantization of x
  w_dq: weight (fp8 values held in fp32) * per-128x128-block scale

Sharding: column-parallel. weight/weight_scale_inv split along N across 8
cores, x replicated; each core computes y[:, c*2048:(c+1)*2048].

Per-core kernel: dequantize both operands to bf16 on-chip (TRN e4m3 max is
240 vs OCP's 448, so x is quantized with scale amax/224 — a power-of-two
rescale of the reference's amax/448 grid, giving identical rounding), then
a k-on-partitions bf16 GEMM with fp32 PSUM accumulation.

The PE runs only GEMM matmuls (~884us roofline for this shape): weight
tiles are transposed by XBAR dma_start_transpose instead of the PE, all
on the sync ring (transposes from two HWDGE rings concurrently corrupt
data on HW).  Weight dequant-scale mults alternate between gpsimd and
vector (gpsimd alone is ~2.5x slower than the HBM delivers pieces).  The
first two m-tiles run chunk-major at half-K granularity so the matmul
stream starts early and stays dense (keeps HAM warm) while the 32MB
weight read streams in; all 8 PSUM banks rotate through 512-wide
accumulation groups.
"""

import numpy as np

M, K, N = 4096, 4096, 16384
NCORES = 8
NSH = N // NCORES          # 2048 columns of y per core
P = 128
KB = K // P                # 32 k-blocks
NB = NSH // P              # 16 n-blocks per core
FP8_SAFE = 224.0           # 448/2: fits TRN e4m3 (max 240), same rounding grid

_NC_CACHE = {}


def _build(M=M, K=K, NSH=NSH, debug=False):
    import concourse.bass as bass  # noqa: F401
    import concourse.mybir as mybir
    import concourse.tile as tile
    from concourse import bacc

    KB = K // P                # k-blocks
    KH = KB // 2               # k-blocks per half
    MT = M // P                # m-tiles
    NB = NSH // P              # n-blocks
    CHW = min(512, NSH)        # psum chunk width
    NCH = NSH // CHW           # chunks per core
    NPC = CHW // P             # n-blocks per chunk

    f32, bf16, f8 = mybir.dt.float32, mybir.dt.bfloat16, mybir.dt.float8e4

    nc = bacc.Bacc(None, target_bir_lowering=False, debug=debug)
    x_d = nc.declare_dram_parameter("x", [M, K], f32, isOutput=False)
    w_d = nc.declare_dram_parameter("w", [NSH, K], f32, isOutput=False)
    ws_d = nc.declare_dram_parameter("ws", [NB, KB], f32, isOutput=False)
    y_d = nc.declare_dram_parameter("y", [M, NSH], f32, isOutput=True)

    with tile.TileContext(nc) as tc:
        with (
            tc.tile_pool(name="const", bufs=1) as const,
            tc.tile_pool(name="wt", bufs=1) as wtp,
            tc.tile_pool(name="wdq", bufs=2) as wpool,
            tc.tile_pool(name="xrow", bufs=2) as xpool,
            tc.tile_pool(name="xq", bufs=2) as xqp,
            tc.tile_pool(name="xdq", bufs=2) as xdp,
            tc.tile_pool(name="xt", bufs=8) as xtp,
            tc.tile_pool(name="scales", bufs=3) as spool,
            tc.tile_pool(name="ypool", bufs=3) as ypool,
            tc.tile_pool(name="psum", bufs=8, space="PSUM") as psum,
        ):
            # ---- weight-block scales, broadcast to all partitions ----
            ws_row = const.tile([1, NB * KB], f32)
            nc.sync.dma_start(ws_row[:], ws_d[:].rearrange("a b -> (a b)")[None, :])
            ws_b = const.tile([P, NB, KB], f32)
            nc.gpsimd.partition_broadcast(
                ws_b[:].rearrange("p a b -> p (a b)"), ws_row[:]
            )

            # Persistent transposed weights: [k-part, nb, kb, n].  For chunk c
            # the matmul streams wT[:, c*NPC:(c+1)*NPC, kb, :] (3D strided AP);
            # each XBAR transpose destination wT[:, nb, khalf, :] is contiguous.
            wT = wtp.tile([P, NB, KB, P], bf16)

            def w_piece(nb, kh):
                # fp32->bf16 cast in-flight (SWDGE). Exact: weight holds
                # fp8-representable values.  Scale mults alternate between
                # gpsimd and vector; transpose on the sync-ring XBAR.
                ks = slice(kh * KH * P, (kh + 1) * KH * P)
                wdq = wpool.tile([P, KH, P], bf16, tag="wdq")
                nc.gpsimd.dma_start(
                    wdq[:],
                    w_d[nb * P:(nb + 1) * P, ks].rearrange(
                        "n (kb x) -> n kb x", x=P
                    ),
                )
                eng = nc.vector if nb % 2 == 0 else nc.gpsimd
                eng.tensor_tensor(
                    wdq[:], wdq[:],
                    ws_b[:, nb, kh * KH:(kh + 1) * KH, None].to_broadcast(
                        (P, KH, P)
                    ),
                    mybir.AluOpType.mult,
                )
                nc.sync.dma_start_transpose(
                    wT[:, nb, kh * KH:(kh + 1) * KH, :],
                    wdq[:].rearrange("p a b -> p (a b)"),
                )

            xts = {}

            def x_prep(mt):
                # quantize+dequantize one m-tile of x (two k-halves) onto the
                # reference fp8 grid, then XBAR-transpose to k-on-partitions.
                ms = slice(mt * P, (mt + 1) * P)
                sc = spool.tile([P, 3, KB], f32, tag="sc")
                amax, rinv, s2 = sc[:, 0, :], sc[:, 1, :], sc[:, 2, :]
                xrows = []
                for kh in range(2):
                    ks = slice(kh * KH * P, (kh + 1) * KH * P)
                    xrow = xpool.tile([P, KH, P], f32, tag="xrow")
                    nc.sync.dma_start(
                        xrow[:],
                        x_d[ms, ks].rearrange("m (kb x) -> m kb x", x=P),
                    )
                    nc.vector.tensor_reduce(
                        amax[:, kh * KH:(kh + 1) * KH], xrow[:],
                        axis=mybir.AxisListType.X,
                        op=mybir.AluOpType.max, apply_absolute_value=True,
                    )
                    xrows.append(xrow)
                nc.vector.reciprocal(rinv, amax)
                nc.vector.tensor_scalar_mul(rinv, rinv, float(FP8_SAFE))
                nc.vector.tensor_scalar_mul(s2, amax, float(1.0 / FP8_SAFE))
                halves = []
                for kh in range(2):
                    khs = slice(kh * KH, (kh + 1) * KH)
                    xq = xqp.tile([P, KH, P], f8, tag="xq")
                    nc.vector.tensor_tensor(
                        xq[:], xrows[kh][:],
                        rinv[:, khs, None].to_broadcast((P, KH, P)),
                        mybir.AluOpType.mult,
                    )
                    xdq = xdp.tile([P, KH, P], bf16, tag="xdq")
                    nc.vector.tensor_tensor(
                        xdq[:], xq[:],
                        s2[:, khs, None].to_broadcast((P, KH, P)),
                        mybir.AluOpType.mult,
                    )
                    # per-half xT tiles: kh0 matmuls depend only on the h0
                    # transpose, not the whole m-tile (tile-granular waits
                    # otherwise delay the first matmuls by ~10us).
                    xTh = xtp.tile([P, KH, P], bf16, tag="xT")
                    nc.sync.dma_start_transpose(
                        xTh[:], xdq[:].rearrange("p a b -> p (a b)")
                    )
                    halves.append(xTh)
                xts[mt] = halves

            pts = {}

            def half_job(mt, c, kh):
                # 16 matmuls: psum[mt,c] += x[mt, khalf].T @ w[chunk c, khalf]
                if kh == 0:
                    pts[(mt, c)] = psum.tile(
                        [P, CHW], mybir.dt.float32, name=f"pt{mt}_{c}", tag="pt"
                    )
                pt = pts[(mt, c)]
                xTh = xts[mt][kh]
                for kb in range(kh * KH, (kh + 1) * KH):
                    nc.tensor.matmul(
                        pt[:],
                        xTh[:, kb - kh * KH, :],
                        wT[:, c * NPC:(c + 1) * NPC, kb, :],
                        start=(kb == 0),
                        stop=(kb == KB - 1),
                    )

            def drain(mt, c):
                pt = pts.pop((mt, c))
                yt = ypool.tile([P, CHW], mybir.dt.float32, tag="yt")
                if c % 2 == 0:
                    nc.scalar.activation(
                        yt[:], pt[:], mybir.ActivationFunctionType.Copy
                    )
                else:
                    nc.vector.tensor_copy(yt[:], pt[:])
                nc.scalar.dma_start(
                    y_d[mt * P:(mt + 1) * P, c * CHW:(c + 1) * CHW], yt[:]
                )

            # ---- emission ----
            # Early weights (chunk 0 first half) so matmuls can start ASAP,
            # interleaved with x-prep for the ramp m-tiles.
            R = min(2, MT)          # ramp m-tiles, processed chunk-major
            NPREP = min(4, MT)      # m-tiles prepped ahead of the job stream
            for nb in range(min(NPC, NB)):
                w_piece(nb, 0)
            for mt in range(min(2, NPREP)):
                x_prep(mt)
            for nb in range(min(NPC, NB)):
                w_piece(nb, 1)
            for mt in range(2, NPREP):
                x_prep(mt)
            for c in range(1, NCH):
                for kh in range(2):
                    for nb in range(c * NPC, (c + 1) * NPC):
                        w_piece(nb, kh)

            # Ramp: chunk-major at half-K granularity over the first R m-tiles.
            for c in range(NCH):
                for kh in range(2):
                    for mt in range(R):
                        half_job(mt, c, kh)
                        if kh == 1:
                            drain(mt, c)

            # Steady state: m-tile-major.
            for mt in range(R, MT):
                if mt + 2 < MT and mt + 2 >= NPREP:
                    x_prep(mt + 2)
                for c in range(NCH):
                    for kh in range(2):
                        half_job(mt, c, kh)
                    drain(mt, c)

    nc.compile()
    return nc


def kernel(x, weight, weight_scale_inv):
    from concourse.bass_utils import run_bass_kernel_spmd

    if "nc" not in _NC_CACHE:
        _NC_CACHE["nc"] = _build()
    nc = _NC_CACHE["nc"]

    x = np.ascontiguousarray(np.asarray(x, dtype=np.float32))
    weight = np.asarray(weight, dtype=np.float32)
    ws = np.asarray(weight_scale_inv, dtype=np.float32)

    in_maps = [
        {
            "x": x,
            "w": np.ascontiguousarray(weight[c * NSH:(c + 1) * NSH]),
            "ws": np.ascontiguousarray(ws[c * NB:(c + 1) * NB]),
        }
        for c in range(NCORES)
    ]
    res = run_bass_kernel_spmd(nc, in_maps, list(range(NCORES)))
    y = np.concatenate([res.results[c]["y"] for c in range(NCORES)], axis=1)
    return y.astype(np.float32, copy=False)
